# revision 3
# baseline (speedup 1.0000x reference)
"""Banded sparse attention + MLP projections for TRN2, 8-core SPMD.

Problem: out = (softmax(mask(Q K^T / sqrt(dk))) V) W_O + b_O with
Q/K/V = x W_{Q,K,V} + b, x:[4, 2048, 512], 8 heads, dk=64.

The "log-sparse + k neighbors" mask with k = S//2 = 1024 degenerates to a
banded causal mask: valid iff 0 <= i - j <= 1024 (powers of 2 above 1024
exceed the max distance 2047... the next power is 2048 which is out of
range).  So each 128-query tile attends to at most 9 key tiles.

Sharding: 8 cores = 4 batches x 2 head-groups (4 heads each).  Each core
computes its heads' Q^T/K^T/V projections, banded attention in a
scores-transposed layout (kpos on partitions), and a partial O-projection
outT = W_O[heads].T @ attn_out^T of shape [512, 2048].  Host sums the two
half-partials per batch, transposes, and adds b_O.

All matmuls run in bf16 (fp32 PSUM accumulation); measured end-to-end
scale-relative absmax error vs the fp32 reference is ~3e-3.
"""

import functools
from contextlib import ExitStack

import numpy as np
import ml_dtypes

import concourse.bacc as bacc
import concourse.mybir as mybir
import concourse.tile as tile
from concourse.bass_utils import run_bass_kernel_spmd
from concourse.masks import make_identity, make_upper_triangular, make_lower_triangular

BF16 = mybir.dt.bfloat16
F32 = mybir.dt.float32
NBF = ml_dtypes.bfloat16

S, D = 2048, 512
NT = S // 128          # 16 token tiles
MAXNK = 9              # max key tiles in the band per query tile
N_CORES = 8

LAST_RESULTS = None    # BassKernelResults of the most recent run (for profiling)


def _emit(ctx: ExitStack, tc, io):
    nc = tc.nc
    xT, wq, wk, wv, wo, bq, bk, bv, outT = (
        io[k] for k in ("xT", "wq", "wk", "wv", "wo", "bq", "bk", "bv", "outT")
    )

    persist = ctx.enter_context(tc.tile_pool(name="persist", bufs=1))

    ident = persist.tile([128, 128], BF16)
    make_identity(nc, ident)
    # scores are held transposed: [kpos (partition), q (free)].
    # diag tile valid iff q >= k  -> upper triangular incl diag
    # left band-edge tile valid iff q <= k -> lower triangular incl diag
    m_diag = persist.tile([128, 128], BF16)
    make_upper_triangular(nc, m_diag, val=1.0, diag=True)
    m_left = persist.tile([128, 128], BF16)
    make_lower_triangular(nc, m_left, val=1.0, diag=True)
    ones_row = persist.tile([1, 512], BF16)
    nc.vector.memset(ones_row, 1.0)

    xT_sb = persist.tile([128, 4, S], BF16)
    wq_sb = persist.tile([128, 4, 256], BF16)
    wk_sb = persist.tile([128, 4, 256], BF16)
    wv_sb = persist.tile([128, 4, 256], BF16)
    for kt in range(4):
        nc.sync.dma_start(out=xT_sb[:, kt, :], in_=xT[kt * 128:(kt + 1) * 128, :])
        nc.sync.dma_start(out=wq_sb[:, kt, :], in_=wq[kt * 128:(kt + 1) * 128, :])
        nc.sync.dma_start(out=wk_sb[:, kt, :], in_=wk[kt * 128:(kt + 1) * 128, :])
        nc.sync.dma_start(out=wv_sb[:, kt, :], in_=wv[kt * 128:(kt + 1) * 128, :])
    wo_sb = persist.tile([128, 2, 512], BF16)
    for pr in range(2):
        nc.sync.dma_start(out=wo_sb[:, pr, :], in_=wo[pr * 128:(pr + 1) * 128, :])
    bq_sb = persist.tile([1, 256], BF16)
    bk_sb = persist.tile([1, 256], BF16)
    bv_sb = persist.tile([1, 256], BF16)
    nc.sync.dma_start(out=bq_sb, in_=bq[:, :])
    nc.sync.dma_start(out=bk_sb, in_=bk[:, :])
    nc.sync.dma_start(out=bv_sb, in_=bv[:, :])

    # Q^T / K^T per head pair: rows 0-63 head A dims, 64-127 head B dims.
    QT_sb = persist.tile([128, 2, S], BF16)
    KT_sb = persist.tile([128, 2, S], BF16)
    # V in [token, d] layout per k-tile, stored as [dA0..dA63, onesA,
    # dB0..dB63, onesB] so [V_h | ones] is one contiguous [128, 65] slice.
    V_sb = persist.tile([128, 2, NT, 130], BF16)
    nc.gpsimd.memset(V_sb[:, :, :, 64:65], 1.0)
    nc.gpsimd.memset(V_sb[:, :, :, 129:130], 1.0)
    # normalized attention output, transposed: rows = head dims of the pair
    OT_sb = persist.tile([128, 2, S], BF16)

    # ---------------- phase 1: projections ----------------
    with tc.tile_pool(name="pj", bufs=3, space="PSUM") as pj:
        for pr in range(2):
            for w_sb, b_sb, dst in ((wq_sb, bq_sb, QT_sb), (wk_sb, bk_sb, KT_sb)):
                for ch in range(4):
                    ps = pj.tile([128, 512], F32, tag="pjq")
                    for kt in range(4):
                        nc.tensor.matmul(
                            ps,
                            lhsT=w_sb[:, kt, pr * 128:(pr + 1) * 128],
                            rhs=xT_sb[:, kt, ch * 512:(ch + 1) * 512],
                            start=(kt == 0),
                            stop=False,
                        )
                    # bias as a K=1 rank-1 update: b[m] * ones[n]
                    nc.tensor.matmul(
                        ps,
                        lhsT=b_sb[:, pr * 128:(pr + 1) * 128],
                        rhs=ones_row,
                        start=False,
                        stop=True,
                    )
                    nc.vector.tensor_copy(
                        out=dst[:, pr, ch * 512:(ch + 1) * 512], in_=ps
                    )
        for tt in range(NT):
            for pr in range(2):
                ps = pj.tile([128, 128], F32, tag="pjv")
                for kt in range(4):
                    nc.tensor.matmul(
                        ps,
                        lhsT=xT_sb[:, kt, tt * 128:(tt + 1) * 128],
                        rhs=wv_sb[:, kt, pr * 128:(pr + 1) * 128],
                        start=(kt == 0),
                        stop=False,
                    )
                nc.tensor.matmul(
                    ps,
                    lhsT=ones_row[:, 0:128],
                    rhs=bv_sb[:, pr * 128:(pr + 1) * 128],
                    start=False,
                    stop=True,
                )
                # interleave the two heads' halves into the 65-col blocks
                src = ps.rearrange("p (two d) -> p two d", two=2)
                dstv = V_sb[:, pr, tt, 0:130].rearrange(
                    "p (two dp) -> p two dp", two=2
                )[:, :, 0:64]
                nc.vector.tensor_copy(out=dstv, in_=src)

    # ---------------- phase 2: banded attention ----------------
    with (
        tc.tile_pool(name="sc", bufs=2, space="PSUM") as scp,
        tc.tile_pool(name="sm", bufs=2, space="PSUM") as smp,
        tc.tile_pool(name="pt", bufs=3) as ptp,
        tc.tile_pool(name="os", bufs=4) as osp,
    ):
        for pr in range(2):
            for qt in range(NT):
                nk = min(qt + 1, MAXNK)
                kt0 = qt - nk + 1
                ps_s = [
                    scp.tile([128, nk * 128], F32, tag="s", name=f"s{h2}")
                    for h2 in range(2)
                ]
                # scores^T: stationary K^T k-tile, moving Q^T q-tile.
                # h2=0 uses array rows 0-63, h2=1 rows 64-127 (concurrent).
                for j in range(nk):
                    kt = kt0 + j
                    for h2 in range(2):
                        lo, hi = h2 * 64, h2 * 64 + 64
                        nc.tensor.matmul(
                            ps_s[h2][:, j * 128:(j + 1) * 128],
                            lhsT=KT_sb[lo:hi, pr, kt * 128:(kt + 1) * 128],
                            rhs=QT_sb[lo:hi, pr, qt * 128:(qt + 1) * 128],
                            start=True,
                            stop=True,
                        )
                out_sb = osp.tile([128, 128], BF16, tag="ob")
                for h2 in range(2):
                    pT = ptp.tile([128, MAXNK * 128], BF16, tag="pt")
                    nc.scalar.activation(
                        out=pT[:, 0:nk * 128],
                        in_=ps_s[h2][:, 0:nk * 128],
                        func=mybir.ActivationFunctionType.Exp,
                        scale=0.125,  # 1/sqrt(dk)
                    )
                    if qt >= MAXNK - 1:
                        nc.gpsimd.tensor_mul(
                            out=pT[:, 0:128], in0=pT[:, 0:128], in1=m_left
                        )
                    nc.gpsimd.tensor_mul(
                        out=pT[:, (nk - 1) * 128:nk * 128],
                        in0=pT[:, (nk - 1) * 128:nk * 128],
                        in1=m_diag,
                    )
                    # out^T accumulate: stationary p^T tile, moving [V|ones].
                    # col 64 of the result is the softmax denominator.
                    ps_o = smp.tile([128, 65], F32, tag="sm")
                    for j in range(nk):
                        kt = kt0 + j
                        nc.tensor.matmul(
                            ps_o,
                            lhsT=pT[:, j * 128:(j + 1) * 128],
                            rhs=V_sb[:, pr, kt, h2 * 65:(h2 + 1) * 65],
                            start=(j == 0),
                            stop=(j == nk - 1),
                        )
                    recip = osp.tile([128, 1], F32, tag="rc")
                    nc.vector.reciprocal(out=recip, in_=ps_o[:, 64:65])
                    nc.vector.tensor_scalar_mul(
                        out=out_sb[:, h2 * 64:(h2 + 1) * 64],
                        in0=ps_o[:, 0:64],
                        scalar1=recip,
                    )
                ps_t = smp.tile([128, 128], BF16, tag="sm")
                nc.tensor.transpose(out=ps_t, in_=out_sb, identity=ident)
                nc.vector.tensor_copy(
                    out=OT_sb[:, pr, qt * 128:(qt + 1) * 128], in_=ps_t
                )

    # ---------------- phase 3: partial O-projection ----------------
    with (
        tc.tile_pool(name="fo", bufs=2, space="PSUM") as fop,
        tc.tile_pool(name="fs", bufs=2) as fsp,
    ):
        for ot in range(4):
            for ch in range(4):
                ps = fop.tile([128, 512], F32, tag="fo")
                for pr in range(2):
                    nc.tensor.matmul(
                        ps,
                        lhsT=wo_sb[:, pr, ot * 128:(ot + 1) * 128],
                        rhs=OT_sb[:, pr, ch * 512:(ch + 1) * 512],
                        start=(pr == 0),
                        stop=(pr == 1),
                    )
                fs = fsp.tile([128, 512], F32, tag="fs")
                nc.vector.tensor_copy(out=fs, in_=ps)
                nc.sync.dma_start(
                    out=outT[ot * 128:(ot + 1) * 128, ch * 512:(ch + 1) * 512],
                    in_=fs,
                )


@functools.lru_cache(maxsize=1)
def _build():
    nc = bacc.Bacc(
        "TRN2", target_bir_lowering=False, debug=False, num_devices=N_CORES
    )
    io = {
        "xT": nc.dram_tensor("xT", [D, S], BF16, kind="ExternalInput").ap(),
        "wq": nc.dram_tensor("wq", [D, 256], BF16, kind="ExternalInput").ap(),
        "wk": nc.dram_tensor("wk", [D, 256], BF16, kind="ExternalInput").ap(),
        "wv": nc.dram_tensor("wv", [D, 256], BF16, kind="ExternalInput").ap(),
        "wo": nc.dram_tensor("wo", [256, D], BF16, kind="ExternalInput").ap(),
        "bq": nc.dram_tensor("bq", [1, 256], BF16, kind="ExternalInput").ap(),
        "bk": nc.dram_tensor("bk", [1, 256], BF16, kind="ExternalInput").ap(),
        "bv": nc.dram_tensor("bv", [1, 256], BF16, kind="ExternalInput").ap(),
        "outT": nc.dram_tensor("outT", [D, S], F32, kind="ExternalOutput").ap(),
    }
    with tile.TileContext(nc) as tc:
        with ExitStack() as ctx:
            _emit(ctx, tc, io)
    nc.compile()
    return nc


def make_in_maps(x, W_Q, b_Q, W_K, b_K, W_V, b_V, W_O, b_O):
    in_maps = []
    for c in range(N_CORES):
        b, hg = c // 2, c % 2
        hs = hg * 256
        in_maps.append(
            {
                "xT": np.ascontiguousarray(x[b].T).astype(NBF),
                "wq": np.ascontiguousarray(W_Q[:, hs:hs + 256]).astype(NBF),
                "wk": np.ascontiguousarray(W_K[:, hs:hs + 256]).astype(NBF),
                "wv": np.ascontiguousarray(W_V[:, hs:hs + 256]).astype(NBF),
                "wo": np.ascontiguousarray(W_O[hs:hs + 256, :]).astype(NBF),
                "bq": b_Q[None, hs:hs + 256].astype(NBF),
                "bk": b_K[None, hs:hs + 256].astype(NBF),
                "bv": b_V[None, hs:hs + 256].astype(NBF),
            }
        )
    return in_maps


def kernel(x, W_Q, b_Q, W_K, b_K, W_V, b_V, W_O, b_O):
    global LAST_RESULTS
    x, W_Q, b_Q, W_K, b_K, W_V, b_V, W_O, b_O = (
        np.asarray(a, dtype=np.float32)
        for a in (x, W_Q, b_Q, W_K, b_K, W_V, b_V, W_O, b_O)
    )
    nc = _build()
    in_maps = make_in_maps(x, W_Q, b_Q, W_K, b_K, W_V, b_V, W_O, b_O)
    res = run_bass_kernel_spmd(nc, in_maps, core_ids=list(range(N_CORES)))
    LAST_RESULTS = res
    out = np.empty((4, S, D), np.float32)
    for b in range(4):
        acc = res.results[2 * b]["outT"].astype(np.float32) + res.results[
            2 * b + 1
        ]["outT"].astype(np.float32)
        out[b] = acc.T + b_O[None, :]
    return out


# revision 16
# speedup vs baseline: 1.1000x; 1.1000x over previous
"""Banded sparse attention + MLP projections for TRN2, 8-core SPMD.

Problem: out = (softmax(mask(Q K^T / sqrt(dk))) V) W_O + b_O with
Q/K/V = x W_{Q,K,V} + b, x:[4, 2048, 512], 8 heads, dk=64.

The "log-sparse + k neighbors" mask with k = S//2 = 1024 degenerates to a
banded causal mask: valid iff 0 <= i - j <= 1024 (powers of 2 above 1024
exceed the max distance 2047... the next power is 2048 which is out of
range).  So each 128-query tile attends to at most 9 key tiles.

Sharding: 8 cores = 4 batches x 2 head-groups (4 heads each).  Each core
computes its heads' Q^T/K^T/V projections, banded attention in a
scores-transposed layout (kpos on partitions), and a partial O-projection
outT = W_O[heads].T @ attn_out^T of shape [512, 2048].  Host sums the two
half-partials per batch, transposes, and adds b_O.

All matmuls run in bf16 (fp32 PSUM accumulation); measured end-to-end
scale-relative absmax error vs the fp32 reference is ~3e-3.
"""

import functools
from contextlib import ExitStack

import numpy as np
import ml_dtypes

import concourse.bacc as bacc
import concourse.mybir as mybir
import concourse.tile as tile
from concourse.bass_utils import run_bass_kernel_spmd
from concourse.masks import make_identity, make_upper_triangular, make_lower_triangular

BF16 = mybir.dt.bfloat16
F32 = mybir.dt.float32
NBF = ml_dtypes.bfloat16

S, D = 2048, 512
NT = S // 128          # 16 token tiles
MAXNK = 9              # max key tiles in the band per query tile
N_CORES = 8

LAST_RESULTS = None    # BassKernelResults of the most recent run (for profiling)


def _emit(ctx: ExitStack, tc, io, use_bias):
    nc = tc.nc
    xT, wq, wk, wv, wo, bq, bk, bv, outT = (
        io[k] for k in ("xT", "wq", "wk", "wv", "wo", "bq", "bk", "bv", "outT")
    )

    persist = ctx.enter_context(tc.tile_pool(name="persist", bufs=1))

    ident = persist.tile([128, 128], BF16)
    make_identity(nc, ident)
    # scores are held transposed: [kpos (partition), q (free)].
    # diag tile valid iff q >= k  -> upper triangular incl diag
    # left band-edge tile valid iff q <= k -> lower triangular incl diag
    m_diag = persist.tile([128, 128], BF16)
    make_upper_triangular(nc, m_diag, val=1.0, diag=True)
    m_left = persist.tile([128, 128], BF16)
    make_lower_triangular(nc, m_left, val=1.0, diag=True)
    ones_row = persist.tile([1, 512], BF16)
    nc.vector.memset(ones_row, 1.0)

    xT_sb = persist.tile([128, 4, S], BF16)
    wq_sb = persist.tile([128, 4, 256], BF16)
    wk_sb = persist.tile([128, 4, 256], BF16)
    wv_sb = persist.tile([128, 4, 256], BF16)
    for kt in range(4):
        nc.sync.dma_start(out=xT_sb[:, kt, :], in_=xT[kt * 128:(kt + 1) * 128, :])
        nc.sync.dma_start(out=wq_sb[:, kt, :], in_=wq[kt * 128:(kt + 1) * 128, :])
        nc.sync.dma_start(out=wk_sb[:, kt, :], in_=wk[kt * 128:(kt + 1) * 128, :])
        nc.sync.dma_start(out=wv_sb[:, kt, :], in_=wv[kt * 128:(kt + 1) * 128, :])
    wo_sb = persist.tile([128, 2, 512], BF16)
    for pr in range(2):
        nc.sync.dma_start(out=wo_sb[:, pr, :], in_=wo[pr * 128:(pr + 1) * 128, :])
    bq_sb = persist.tile([1, 256], BF16)
    bk_sb = persist.tile([1, 256], BF16)
    bv_sb = persist.tile([1, 256], BF16)
    nc.sync.dma_start(out=bq_sb, in_=bq[:, :])
    nc.sync.dma_start(out=bk_sb, in_=bk[:, :])
    nc.sync.dma_start(out=bv_sb, in_=bv[:, :])

    # Q^T / K^T per head pair: rows 0-63 head A dims, 64-127 head B dims.
    QT_sb = persist.tile([128, 2, S], BF16)
    KT_sb = persist.tile([128, 2, S], BF16)
    # V in [token, d] layout per k-tile, stored as [dA0..dA63, onesA,
    # dB0..dB63, onesB] so [V_h | ones] is one contiguous [128, 65] slice.
    V_sb = persist.tile([128, 2, NT, 130], BF16)
    nc.gpsimd.memset(V_sb[:, :, :, 64:65], 1.0)
    nc.gpsimd.memset(V_sb[:, :, :, 129:130], 1.0)
    # normalized attention output, transposed: rows = head dims of the pair
    OT_sb = persist.tile([128, 2, S], BF16)

    # ---------------- phase 1: projections ----------------
    with tc.tile_pool(name="pj", bufs=3, space="PSUM") as pj:
        for pr in range(2):
            for w_sb, b_sb, dst in ((wq_sb, bq_sb, QT_sb), (wk_sb, bk_sb, KT_sb)):
                for ch in range(4):
                    ps = pj.tile([128, 512], F32, tag="pjq")
                    for kt in range(4):
                        nc.tensor.matmul(
                            ps,
                            lhsT=w_sb[:, kt, pr * 128:(pr + 1) * 128],
                            rhs=xT_sb[:, kt, ch * 512:(ch + 1) * 512],
                            start=(kt == 0),
                            stop=(kt == 3 and not use_bias),
                        )
                    if use_bias:
                        # bias as a K=1 rank-1 update: b[m] * ones[n]
                        nc.tensor.matmul(
                            ps,
                            lhsT=b_sb[:, pr * 128:(pr + 1) * 128],
                            rhs=ones_row,
                            start=False,
                            stop=True,
                        )
                    nc.vector.tensor_copy(
                        out=dst[:, pr, ch * 512:(ch + 1) * 512], in_=ps
                    )
        for tt in range(NT):
            ps = pj.tile([128, 256], F32, tag="pjv")
            for kt in range(4):
                nc.tensor.matmul(
                    ps,
                    lhsT=xT_sb[:, kt, tt * 128:(tt + 1) * 128],
                    rhs=wv_sb[:, kt, 0:256],
                    start=(kt == 0),
                    stop=(kt == 3 and not use_bias),
                )
            if use_bias:
                nc.tensor.matmul(
                    ps,
                    lhsT=ones_row[:, 0:128],
                    rhs=bv_sb[:, 0:256],
                    start=False,
                    stop=True,
                )
            # interleave each pair's head halves into its 65-col blocks
            for pr in range(2):
                src = ps[:, pr * 128:(pr + 1) * 128].rearrange(
                    "p (two d) -> p two d", two=2
                )
                dstv = V_sb[:, pr, tt, 0:130].rearrange(
                    "p (two dp) -> p two dp", two=2
                )[:, :, 0:64]
                nc.vector.tensor_copy(out=dstv, in_=src)

    # ---------------- phase 2: banded attention ----------------
    with (
        tc.tile_pool(name="sc", bufs=2, space="PSUM") as scp,
        tc.tile_pool(name="sm", bufs=2, space="PSUM") as smp,
        tc.tile_pool(name="pt", bufs=4) as ptp,
        tc.tile_pool(name="os", bufs=4) as osp,
    ):
        for pr in range(2):
            for qt in range(NT):
                nk = min(qt + 1, MAXNK)
                kt0 = qt - nk + 1
                ps_s = [
                    scp.tile([128, nk * 128], F32, tag="s", name=f"s{h2}")
                    for h2 in range(2)
                ]
                # scores^T: stationary K^T k-tile, moving Q^T q-tile.
                # h2=0 uses array rows 0-63, h2=1 rows 64-127 (concurrent).
                for j in range(nk):
                    kt = kt0 + j
                    for h2 in range(2):
                        lo, hi = h2 * 64, h2 * 64 + 64
                        nc.tensor.matmul(
                            ps_s[h2][:, j * 128:(j + 1) * 128],
                            lhsT=KT_sb[lo:hi, pr, kt * 128:(kt + 1) * 128],
                            rhs=QT_sb[lo:hi, pr, qt * 128:(qt + 1) * 128],
                            start=True,
                            stop=True,
                        )
                out_sb = osp.tile([128, 128], BF16, tag="ob")
                for h2 in range(2):
                    pT = ptp.tile([128, MAXNK * 128], BF16, tag="pt")
                    nc.scalar.activation(
                        out=pT[:, 0:nk * 128],
                        in_=ps_s[h2][:, 0:nk * 128],
                        func=mybir.ActivationFunctionType.Exp,
                        scale=0.125,  # 1/sqrt(dk)
                    )
                    if qt >= MAXNK - 1:
                        nc.gpsimd.tensor_mul(
                            out=pT[:, 0:128], in0=pT[:, 0:128], in1=m_left
                        )
                    nc.gpsimd.tensor_mul(
                        out=pT[:, (nk - 1) * 128:nk * 128],
                        in0=pT[:, (nk - 1) * 128:nk * 128],
                        in1=m_diag,
                    )
                    # out^T accumulate: stationary p^T tile, moving [V|ones].
                    # col 64 of the result is the softmax denominator.
                    ps_o = smp.tile([128, 65], F32, tag="sm")
                    for j in range(nk):
                        kt = kt0 + j
                        nc.tensor.matmul(
                            ps_o,
                            lhsT=pT[:, j * 128:(j + 1) * 128],
                            rhs=V_sb[:, pr, kt, h2 * 65:(h2 + 1) * 65],
                            start=(j == 0),
                            stop=(j == nk - 1),
                        )
                    recip = osp.tile([128, 1], F32, tag="rc")
                    nc.vector.reciprocal(out=recip, in_=ps_o[:, 64:65])
                    nc.vector.tensor_scalar_mul(
                        out=out_sb[:, h2 * 64:(h2 + 1) * 64],
                        in0=ps_o[:, 0:64],
                        scalar1=recip,
                    )
                ps_t = smp.tile([128, 128], BF16, tag="sm")
                nc.tensor.transpose(out=ps_t, in_=out_sb, identity=ident)
                nc.vector.tensor_copy(
                    out=OT_sb[:, pr, qt * 128:(qt + 1) * 128], in_=ps_t
                )

    # ---------------- phase 3: partial O-projection ----------------
    with (
        tc.tile_pool(name="fo", bufs=2, space="PSUM") as fop,
        tc.tile_pool(name="fs", bufs=2) as fsp,
    ):
        for ot in range(4):
            for ch in range(4):
                ps = fop.tile([128, 512], F32, tag="fo")
                for pr in range(2):
                    nc.tensor.matmul(
                        ps,
                        lhsT=wo_sb[:, pr, ot * 128:(ot + 1) * 128],
                        rhs=OT_sb[:, pr, ch * 512:(ch + 1) * 512],
                        start=(pr == 0),
                        stop=(pr == 1),
                    )
                fs = fsp.tile([128, 512], F32, tag="fs")
                nc.vector.tensor_copy(out=fs, in_=ps)
                nc.sync.dma_start(
                    out=outT[ot * 128:(ot + 1) * 128, ch * 512:(ch + 1) * 512],
                    in_=fs,
                )


@functools.lru_cache(maxsize=2)
def _build(use_bias=True):
    nc = bacc.Bacc(
        "TRN2", target_bir_lowering=False, debug=False, num_devices=N_CORES
    )
    io = {
        "xT": nc.dram_tensor("xT", [D, S], BF16, kind="ExternalInput").ap(),
        "wq": nc.dram_tensor("wq", [D, 256], BF16, kind="ExternalInput").ap(),
        "wk": nc.dram_tensor("wk", [D, 256], BF16, kind="ExternalInput").ap(),
        "wv": nc.dram_tensor("wv", [D, 256], BF16, kind="ExternalInput").ap(),
        "wo": nc.dram_tensor("wo", [256, D], BF16, kind="ExternalInput").ap(),
        "bq": nc.dram_tensor("bq", [1, 256], BF16, kind="ExternalInput").ap(),
        "bk": nc.dram_tensor("bk", [1, 256], BF16, kind="ExternalInput").ap(),
        "bv": nc.dram_tensor("bv", [1, 256], BF16, kind="ExternalInput").ap(),
        "outT": nc.dram_tensor("outT", [D, S], F32, kind="ExternalOutput").ap(),
    }
    with tile.TileContext(nc) as tc:
        with ExitStack() as ctx:
            _emit(ctx, tc, io, use_bias)
    nc.compile()
    return nc


def make_in_maps(x, W_Q, b_Q, W_K, b_K, W_V, b_V, W_O, b_O):
    in_maps = []
    for c in range(N_CORES):
        b, hg = c // 2, c % 2
        hs = hg * 256
        in_maps.append(
            {
                "xT": np.ascontiguousarray(x[b].T).astype(NBF),
                "wq": np.ascontiguousarray(W_Q[:, hs:hs + 256]).astype(NBF),
                "wk": np.ascontiguousarray(W_K[:, hs:hs + 256]).astype(NBF),
                "wv": np.ascontiguousarray(W_V[:, hs:hs + 256]).astype(NBF),
                "wo": np.ascontiguousarray(W_O[hs:hs + 256, :]).astype(NBF),
                "bq": b_Q[None, hs:hs + 256].astype(NBF),
                "bk": b_K[None, hs:hs + 256].astype(NBF),
                "bv": b_V[None, hs:hs + 256].astype(NBF),
            }
        )
    return in_maps


def kernel(x, W_Q, b_Q, W_K, b_K, W_V, b_V, W_O, b_O):
    global LAST_RESULTS
    x, W_Q, b_Q, W_K, b_K, W_V, b_V, W_O, b_O = (
        np.asarray(a, dtype=np.float32)
        for a in (x, W_Q, b_Q, W_K, b_K, W_V, b_V, W_O, b_O)
    )
    use_bias = bool(
        np.any(b_Q) or np.any(b_K) or np.any(b_V)
    )  # projection biases are all-zero in this model's inputs
    nc = _build(use_bias)
    in_maps = make_in_maps(x, W_Q, b_Q, W_K, b_K, W_V, b_V, W_O, b_O)
    res = run_bass_kernel_spmd(nc, in_maps, core_ids=list(range(N_CORES)))
    LAST_RESULTS = res
    out = np.empty((4, S, D), np.float32)
    for b in range(4):
        acc = res.results[2 * b]["outT"].astype(np.float32) + res.results[
            2 * b + 1
        ]["outT"].astype(np.float32)
        out[b] = acc.T + b_O[None, :]
    return out


# revision 18
# speedup vs baseline: 1.2212x; 1.1102x over previous
"""Banded sparse attention + MLP projections for TRN2, 8-core SPMD.

Problem: out = (softmax(mask(Q K^T / sqrt(dk))) V) W_O + b_O with
Q/K/V = x W_{Q,K,V} + b, x:[4, 2048, 512], 8 heads, dk=64.

The "log-sparse + k neighbors" mask with k = S//2 = 1024 degenerates to a
banded causal mask: valid iff 0 <= i - j <= 1024 (powers of 2 above 1024
exceed the max distance 2047... the next power is 2048 which is out of
range).  So each 128-query tile attends to at most 9 key tiles.

Sharding: 8 cores = 4 batches x 2 head-groups (4 heads each).  Each core
computes its heads' Q^T/K^T/V projections, banded attention in a
scores-transposed layout (kpos on partitions), and a partial O-projection
outT = W_O[heads].T @ attn_out^T of shape [512, 2048].  Host sums the two
half-partials per batch, transposes, and adds b_O.

All matmuls run in bf16 (fp32 PSUM accumulation); measured end-to-end
scale-relative absmax error vs the fp32 reference is ~3e-3.
"""

import functools
from contextlib import ExitStack

import numpy as np
import ml_dtypes

import concourse.bacc as bacc
import concourse.mybir as mybir
import concourse.tile as tile
from concourse.bass_utils import run_bass_kernel_spmd
from concourse.masks import make_identity, make_upper_triangular, make_lower_triangular

BF16 = mybir.dt.bfloat16
F32 = mybir.dt.float32
NBF = ml_dtypes.bfloat16

S, D = 2048, 512
NT = S // 128          # 16 token tiles
MAXNK = 9              # max key tiles in the band per query tile
N_CORES = 8

LAST_RESULTS = None    # BassKernelResults of the most recent run (for profiling)


def _emit(ctx: ExitStack, tc, io, use_bias):
    nc = tc.nc
    xT, wq, wk, wv, wo, bq, bk, bv, outT = (
        io[k] for k in ("xT", "wq", "wk", "wv", "wo", "bq", "bk", "bv", "outT")
    )

    persist = ctx.enter_context(tc.tile_pool(name="persist", bufs=1))

    ident = persist.tile([128, 128], BF16)
    make_identity(nc, ident)
    # scores are held transposed: [kpos (partition), q (free)].
    # diag tile valid iff q >= k  -> upper triangular incl diag
    # left band-edge tile valid iff q <= k -> lower triangular incl diag
    m_diag = persist.tile([128, 128], BF16)
    make_upper_triangular(nc, m_diag, val=1.0, diag=True)
    m_left = persist.tile([128, 128], BF16)
    make_lower_triangular(nc, m_left, val=1.0, diag=True)
    ones_row = persist.tile([1, 512], BF16)
    nc.vector.memset(ones_row, 1.0)

    xT_sb = persist.tile([128, 4, S], BF16)
    wq_sb = persist.tile([128, 4, 256], BF16)
    wk_sb = persist.tile([128, 4, 256], BF16)
    wv_sb = persist.tile([128, 4, 256], BF16)
    for kt in range(4):
        nc.sync.dma_start(out=xT_sb[:, kt, :], in_=xT[kt * 128:(kt + 1) * 128, :])
        nc.sync.dma_start(out=wq_sb[:, kt, :], in_=wq[kt * 128:(kt + 1) * 128, :])
        nc.sync.dma_start(out=wk_sb[:, kt, :], in_=wk[kt * 128:(kt + 1) * 128, :])
        nc.sync.dma_start(out=wv_sb[:, kt, :], in_=wv[kt * 128:(kt + 1) * 128, :])
    wo_sb = persist.tile([128, 2, 512], BF16)
    for pr in range(2):
        nc.sync.dma_start(out=wo_sb[:, pr, :], in_=wo[pr * 128:(pr + 1) * 128, :])
    bq_sb = persist.tile([1, 256], BF16)
    bk_sb = persist.tile([1, 256], BF16)
    bv_sb = persist.tile([1, 256], BF16)
    nc.sync.dma_start(out=bq_sb, in_=bq[:, :])
    nc.sync.dma_start(out=bk_sb, in_=bk[:, :])
    nc.sync.dma_start(out=bv_sb, in_=bv[:, :])

    # Q^T / K^T per head pair: rows 0-63 head A dims, 64-127 head B dims.
    QT_sb = persist.tile([128, 2, S], BF16)
    KT_sb = persist.tile([128, 2, S], BF16)
    # V in [token, d] layout per k-tile, stored as [dA0..dA63, onesA,
    # dB0..dB63, onesB] so [V_h | ones] is one contiguous [128, 65] slice.
    V_sb = persist.tile([128, 2, NT, 130], BF16)
    nc.gpsimd.memset(V_sb[:, :, :, 64:65], 1.0)
    nc.gpsimd.memset(V_sb[:, :, :, 129:130], 1.0)
    # normalized attention output, transposed: rows = head dims of the pair
    OT_sb = persist.tile([128, 2, S], BF16)

    # ---------------- phase 1: projections ----------------
    with tc.tile_pool(name="pj", bufs=4, space="PSUM") as pj:
        for pr in range(2):
            for w_sb, b_sb, dst in ((wq_sb, bq_sb, QT_sb), (wk_sb, bk_sb, KT_sb)):
                for ch in range(4):
                    ps = pj.tile([128, 512], F32, tag="pjq")
                    for kt in range(4):
                        nc.tensor.matmul(
                            ps,
                            lhsT=w_sb[:, kt, pr * 128:(pr + 1) * 128],
                            rhs=xT_sb[:, kt, ch * 512:(ch + 1) * 512],
                            start=(kt == 0),
                            stop=(kt == 3 and not use_bias),
                        )
                    if use_bias:
                        # bias as a K=1 rank-1 update: b[m] * ones[n]
                        nc.tensor.matmul(
                            ps,
                            lhsT=b_sb[:, pr * 128:(pr + 1) * 128],
                            rhs=ones_row,
                            start=False,
                            stop=True,
                        )
                    nc.vector.tensor_copy(
                        out=dst[:, pr, ch * 512:(ch + 1) * 512], in_=ps
                    )
        for tt in range(NT):
            ps = pj.tile([128, 256], F32, tag="pjv")
            for kt in range(4):
                nc.tensor.matmul(
                    ps,
                    lhsT=xT_sb[:, kt, tt * 128:(tt + 1) * 128],
                    rhs=wv_sb[:, kt, 0:256],
                    start=(kt == 0),
                    stop=(kt == 3 and not use_bias),
                )
            if use_bias:
                nc.tensor.matmul(
                    ps,
                    lhsT=ones_row[:, 0:128],
                    rhs=bv_sb[:, 0:256],
                    start=False,
                    stop=True,
                )
            # interleave each pair's head halves into its 65-col blocks
            for pr in range(2):
                src = ps[:, pr * 128:(pr + 1) * 128].rearrange(
                    "p (two d) -> p two d", two=2
                )
                dstv = V_sb[:, pr, tt, 0:130].rearrange(
                    "p (two dp) -> p two dp", two=2
                )[:, :, 0:64]
                nc.vector.tensor_copy(out=dstv, in_=src)

    # ---------------- phase 2: banded attention ----------------
    with (
        tc.tile_pool(name="sc", bufs=2, space="PSUM") as scp,
        tc.tile_pool(name="sm", bufs=2, space="PSUM") as smp,
        tc.tile_pool(name="pt", bufs=4) as ptp,
        tc.tile_pool(name="os", bufs=4) as osp,
    ):
        for pr in range(2):
            for qt in range(NT):
                nk = min(qt + 1, MAXNK)
                kt0 = qt - nk + 1
                ps_s = [
                    scp.tile([128, nk * 128], F32, tag="s", name=f"s{h2}")
                    for h2 in range(2)
                ]
                # scores^T: stationary K^T k-tile, moving Q^T q-tile.
                # h2=0 uses array rows 0-63, h2=1 rows 64-127 (concurrent).
                for j in range(nk):
                    kt = kt0 + j
                    for h2 in range(2):
                        lo, hi = h2 * 64, h2 * 64 + 64
                        nc.tensor.matmul(
                            ps_s[h2][:, j * 128:(j + 1) * 128],
                            lhsT=KT_sb[lo:hi, pr, kt * 128:(kt + 1) * 128],
                            rhs=QT_sb[lo:hi, pr, qt * 128:(qt + 1) * 128],
                            start=True,
                            stop=True,
                        )
                out_sb = osp.tile([128, 128], BF16, tag="ob")
                for h2 in range(2):
                    pT = ptp.tile([128, MAXNK * 128], BF16, tag="pt")
                    nc.scalar.activation(
                        out=pT[:, 0:nk * 128],
                        in_=ps_s[h2][:, 0:nk * 128],
                        func=mybir.ActivationFunctionType.Exp,
                        scale=0.125,  # 1/sqrt(dk)
                    )
                    if qt >= MAXNK - 1:
                        nc.gpsimd.tensor_mul(
                            out=pT[:, 0:128], in0=pT[:, 0:128], in1=m_left
                        )
                    nc.gpsimd.tensor_mul(
                        out=pT[:, (nk - 1) * 128:nk * 128],
                        in0=pT[:, (nk - 1) * 128:nk * 128],
                        in1=m_diag,
                    )
                    # out^T accumulate: stationary p^T tile, moving [V|ones].
                    # col 64 of the result is the softmax denominator.
                    ps_o = smp.tile([128, 65], F32, tag="sm")
                    for j in range(nk):
                        kt = kt0 + j
                        nc.tensor.matmul(
                            ps_o,
                            lhsT=pT[:, j * 128:(j + 1) * 128],
                            rhs=V_sb[:, pr, kt, h2 * 65:(h2 + 1) * 65],
                            start=(j == 0),
                            stop=(j == nk - 1),
                        )
                    recip = osp.tile([128, 1], F32, tag="rc")
                    nc.vector.reciprocal(out=recip, in_=ps_o[:, 64:65])
                    nc.vector.tensor_scalar_mul(
                        out=out_sb[:, h2 * 64:(h2 + 1) * 64],
                        in0=ps_o[:, 0:64],
                        scalar1=recip,
                    )
                ps_t = smp.tile([128, 128], BF16, tag="sm")
                nc.tensor.transpose(out=ps_t, in_=out_sb, identity=ident)
                nc.vector.tensor_copy(
                    out=OT_sb[:, pr, qt * 128:(qt + 1) * 128], in_=ps_t
                )

    # ---------------- phase 3: partial O-projection ----------------
    with (
        tc.tile_pool(name="fo", bufs=4, space="PSUM") as fop,
        tc.tile_pool(name="fs", bufs=4) as fsp,
    ):
        for ot in range(4):
            for ch in range(4):
                ps = fop.tile([128, 512], F32, tag="fo")
                for pr in range(2):
                    nc.tensor.matmul(
                        ps,
                        lhsT=wo_sb[:, pr, ot * 128:(ot + 1) * 128],
                        rhs=OT_sb[:, pr, ch * 512:(ch + 1) * 512],
                        start=(pr == 0),
                        stop=(pr == 1),
                    )
                fs = fsp.tile([128, 512], F32, tag="fs")
                nc.vector.tensor_copy(out=fs, in_=ps)
                nc.sync.dma_start(
                    out=outT[ot * 128:(ot + 1) * 128, ch * 512:(ch + 1) * 512],
                    in_=fs,
                )


@functools.lru_cache(maxsize=2)
def _build(use_bias=True):
    nc = bacc.Bacc(
        "TRN2", target_bir_lowering=False, debug=False, num_devices=N_CORES
    )
    io = {
        "xT": nc.dram_tensor("xT", [D, S], BF16, kind="ExternalInput").ap(),
        "wq": nc.dram_tensor("wq", [D, 256], BF16, kind="ExternalInput").ap(),
        "wk": nc.dram_tensor("wk", [D, 256], BF16, kind="ExternalInput").ap(),
        "wv": nc.dram_tensor("wv", [D, 256], BF16, kind="ExternalInput").ap(),
        "wo": nc.dram_tensor("wo", [256, D], BF16, kind="ExternalInput").ap(),
        "bq": nc.dram_tensor("bq", [1, 256], BF16, kind="ExternalInput").ap(),
        "bk": nc.dram_tensor("bk", [1, 256], BF16, kind="ExternalInput").ap(),
        "bv": nc.dram_tensor("bv", [1, 256], BF16, kind="ExternalInput").ap(),
        "outT": nc.dram_tensor("outT", [D, S], F32, kind="ExternalOutput").ap(),
    }
    with tile.TileContext(nc) as tc:
        with ExitStack() as ctx:
            _emit(ctx, tc, io, use_bias)
    nc.compile()
    return nc


def make_in_maps(x, W_Q, b_Q, W_K, b_K, W_V, b_V, W_O, b_O):
    in_maps = []
    for c in range(N_CORES):
        b, hg = c // 2, c % 2
        hs = hg * 256
        in_maps.append(
            {
                "xT": np.ascontiguousarray(x[b].T).astype(NBF),
                "wq": np.ascontiguousarray(W_Q[:, hs:hs + 256]).astype(NBF),
                "wk": np.ascontiguousarray(W_K[:, hs:hs + 256]).astype(NBF),
                "wv": np.ascontiguousarray(W_V[:, hs:hs + 256]).astype(NBF),
                "wo": np.ascontiguousarray(W_O[hs:hs + 256, :]).astype(NBF),
                "bq": b_Q[None, hs:hs + 256].astype(NBF),
                "bk": b_K[None, hs:hs + 256].astype(NBF),
                "bv": b_V[None, hs:hs + 256].astype(NBF),
            }
        )
    return in_maps


def kernel(x, W_Q, b_Q, W_K, b_K, W_V, b_V, W_O, b_O):
    global LAST_RESULTS
    x, W_Q, b_Q, W_K, b_K, W_V, b_V, W_O, b_O = (
        np.asarray(a, dtype=np.float32)
        for a in (x, W_Q, b_Q, W_K, b_K, W_V, b_V, W_O, b_O)
    )
    use_bias = bool(
        np.any(b_Q) or np.any(b_K) or np.any(b_V)
    )  # projection biases are all-zero in this model's inputs
    nc = _build(use_bias)
    in_maps = make_in_maps(x, W_Q, b_Q, W_K, b_K, W_V, b_V, W_O, b_O)
    res = run_bass_kernel_spmd(nc, in_maps, core_ids=list(range(N_CORES)))
    LAST_RESULTS = res
    out = np.empty((4, S, D), np.float32)
    for b in range(4):
        acc = res.results[2 * b]["outT"].astype(np.float32) + res.results[
            2 * b + 1
        ]["outT"].astype(np.float32)
        out[b] = acc.T + b_O[None, :]
    return out


# revision 20
# speedup vs baseline: 1.2401x; 1.0155x over previous
"""Banded sparse attention + MLP projections for TRN2, 8-core SPMD.

Problem: out = (softmax(mask(Q K^T / sqrt(dk))) V) W_O + b_O with
Q/K/V = x W_{Q,K,V} + b, x:[4, 2048, 512], 8 heads, dk=64.

The "log-sparse + k neighbors" mask with k = S//2 = 1024 degenerates to a
banded causal mask: valid iff 0 <= i - j <= 1024 (powers of 2 above 1024
exceed the max distance 2047... the next power is 2048 which is out of
range).  So each 128-query tile attends to at most 9 key tiles.

Sharding: 8 cores = 4 batches x 2 head-groups (4 heads each).  Each core
computes its heads' Q^T/K^T/V projections, banded attention in a
scores-transposed layout (kpos on partitions), and a partial O-projection
outT = W_O[heads].T @ attn_out^T of shape [512, 2048].  Host sums the two
half-partials per batch, transposes, and adds b_O.

All matmuls run in bf16 (fp32 PSUM accumulation); measured end-to-end
scale-relative absmax error vs the fp32 reference is ~3e-3.
"""

import functools
from contextlib import ExitStack

import numpy as np
import ml_dtypes

import concourse.bacc as bacc
import concourse.mybir as mybir
import concourse.tile as tile
from concourse.bass_utils import run_bass_kernel_spmd
from concourse.masks import make_identity, make_upper_triangular, make_lower_triangular

BF16 = mybir.dt.bfloat16
F32 = mybir.dt.float32
NBF = ml_dtypes.bfloat16

S, D = 2048, 512
NT = S // 128          # 16 token tiles
MAXNK = 9              # max key tiles in the band per query tile
N_CORES = 8

LAST_RESULTS = None    # BassKernelResults of the most recent run (for profiling)


def _emit(ctx: ExitStack, tc, io, use_bias):
    nc = tc.nc
    xT, wq, wk, wv, wo, bq, bk, bv, outT = (
        io[k] for k in ("xT", "wq", "wk", "wv", "wo", "bq", "bk", "bv", "outT")
    )

    persist = ctx.enter_context(tc.tile_pool(name="persist", bufs=1))

    ident = persist.tile([128, 128], BF16)
    make_identity(nc, ident)
    # scores are held transposed: [kpos (partition), q (free)].
    # diag tile valid iff q >= k  -> upper triangular incl diag
    # left band-edge tile valid iff q <= k -> lower triangular incl diag
    m_diag = persist.tile([128, 128], BF16)
    make_upper_triangular(nc, m_diag, val=1.0, diag=True)
    m_left = persist.tile([128, 128], BF16)
    make_lower_triangular(nc, m_left, val=1.0, diag=True)
    ones_row = persist.tile([1, 512], BF16)
    nc.vector.memset(ones_row, 1.0)

    xT_sb = persist.tile([128, 4, S], BF16)
    wq_sb = persist.tile([128, 4, 256], BF16)
    wk_sb = persist.tile([128, 4, 256], BF16)
    wv_sb = persist.tile([128, 4, 256], BF16)
    for kt in range(4):
        nc.sync.dma_start(out=xT_sb[:, kt, :], in_=xT[kt * 128:(kt + 1) * 128, :])
        nc.sync.dma_start(out=wq_sb[:, kt, :], in_=wq[kt * 128:(kt + 1) * 128, :])
        nc.sync.dma_start(out=wk_sb[:, kt, :], in_=wk[kt * 128:(kt + 1) * 128, :])
        nc.sync.dma_start(out=wv_sb[:, kt, :], in_=wv[kt * 128:(kt + 1) * 128, :])
    wo_sb = persist.tile([128, 2, 512], BF16)
    for pr in range(2):
        nc.sync.dma_start(out=wo_sb[:, pr, :], in_=wo[pr * 128:(pr + 1) * 128, :])
    bq_sb = persist.tile([1, 256], BF16)
    bk_sb = persist.tile([1, 256], BF16)
    bv_sb = persist.tile([1, 256], BF16)
    nc.sync.dma_start(out=bq_sb, in_=bq[:, :])
    nc.sync.dma_start(out=bk_sb, in_=bk[:, :])
    nc.sync.dma_start(out=bv_sb, in_=bv[:, :])

    # Q^T / K^T per head pair: rows 0-63 head A dims, 64-127 head B dims.
    QT_sb = persist.tile([128, 2, S], BF16)
    KT_sb = persist.tile([128, 2, S], BF16)
    # V in [token, d] layout per k-tile, stored as [dA0..dA63, onesA,
    # dB0..dB63, onesB] so [V_h | ones] is one contiguous [128, 65] slice.
    V_sb = persist.tile([128, 2, NT, 130], BF16)
    nc.gpsimd.memset(V_sb[:, :, :, 64:65], 1.0)
    nc.gpsimd.memset(V_sb[:, :, :, 129:130], 1.0)
    # normalized attention output, transposed: rows = head dims of the pair
    OT_sb = persist.tile([128, 2, S], BF16)

    # ---------------- phase 1: projections ----------------
    with tc.tile_pool(name="pj", bufs=4, space="PSUM") as pj:
        for pr in range(2):
            for w_sb, b_sb, dst in ((wq_sb, bq_sb, QT_sb), (wk_sb, bk_sb, KT_sb)):
                for ch in range(4):
                    ps = pj.tile([128, 512], F32, tag="pjq")
                    for kt in range(4):
                        nc.tensor.matmul(
                            ps,
                            lhsT=w_sb[:, kt, pr * 128:(pr + 1) * 128],
                            rhs=xT_sb[:, kt, ch * 512:(ch + 1) * 512],
                            start=(kt == 0),
                            stop=(kt == 3 and not use_bias),
                        )
                    if use_bias:
                        # bias as a K=1 rank-1 update: b[m] * ones[n]
                        nc.tensor.matmul(
                            ps,
                            lhsT=b_sb[:, pr * 128:(pr + 1) * 128],
                            rhs=ones_row,
                            start=False,
                            stop=True,
                        )
                    nc.vector.tensor_copy(
                        out=dst[:, pr, ch * 512:(ch + 1) * 512], in_=ps
                    )
        for tt in range(NT):
            ps = pj.tile([128, 256], F32, tag="pjv")
            for kt in range(4):
                nc.tensor.matmul(
                    ps,
                    lhsT=xT_sb[:, kt, tt * 128:(tt + 1) * 128],
                    rhs=wv_sb[:, kt, 0:256],
                    start=(kt == 0),
                    stop=(kt == 3 and not use_bias),
                )
            if use_bias:
                nc.tensor.matmul(
                    ps,
                    lhsT=ones_row[:, 0:128],
                    rhs=bv_sb[:, 0:256],
                    start=False,
                    stop=True,
                )
            # interleave each pair's head halves into its 65-col blocks
            for pr in range(2):
                src = ps[:, pr * 128:(pr + 1) * 128].rearrange(
                    "p (two d) -> p two d", two=2
                )
                dstv = V_sb[:, pr, tt, 0:130].rearrange(
                    "p (two dp) -> p two dp", two=2
                )[:, :, 0:64]
                nc.vector.tensor_copy(out=dstv, in_=src)

    # ---------------- phase 2: banded attention ----------------
    with (
        tc.tile_pool(name="sc", bufs=2, space="PSUM") as scp,
        tc.tile_pool(name="sm", bufs=2, space="PSUM") as smp,
        tc.tile_pool(name="pt", bufs=4) as ptp,
        tc.tile_pool(name="os", bufs=4) as osp,
    ):
        for pr in range(2):
            for qt in range(NT):
                nk = min(qt + 1, MAXNK)
                kt0 = qt - nk + 1
                ps_s = [
                    scp.tile([128, nk * 128], F32, tag="s", name=f"s{h2}")
                    for h2 in range(2)
                ]
                # scores^T: stationary K^T k-tile, moving Q^T q-tile.
                # h2=0 uses array rows 0-63, h2=1 rows 64-127 (concurrent).
                for j in range(nk):
                    kt = kt0 + j
                    for h2 in range(2):
                        lo, hi = h2 * 64, h2 * 64 + 64
                        nc.tensor.matmul(
                            ps_s[h2][:, j * 128:(j + 1) * 128],
                            lhsT=KT_sb[lo:hi, pr, kt * 128:(kt + 1) * 128],
                            rhs=QT_sb[lo:hi, pr, qt * 128:(qt + 1) * 128],
                            start=True,
                            stop=True,
                        )
                out_sb = osp.tile([128, 128], BF16, tag="ob")
                for h2 in range(2):
                    pT = ptp.tile([128, MAXNK * 128], BF16, tag="pt")
                    nc.scalar.activation(
                        out=pT[:, 0:nk * 128],
                        in_=ps_s[h2][:, 0:nk * 128],
                        func=mybir.ActivationFunctionType.Exp,
                        scale=0.125,  # 1/sqrt(dk)
                    )
                    if qt >= MAXNK - 1:
                        nc.gpsimd.tensor_mul(
                            out=pT[:, 0:128], in0=pT[:, 0:128], in1=m_left
                        )
                    nc.gpsimd.tensor_mul(
                        out=pT[:, (nk - 1) * 128:nk * 128],
                        in0=pT[:, (nk - 1) * 128:nk * 128],
                        in1=m_diag,
                    )
                    # out^T accumulate: stationary p^T tile, moving [V|ones].
                    # col 64 of the result is the softmax denominator.
                    ps_o = smp.tile([128, 65], F32, tag="sm")
                    for j in range(nk):
                        kt = kt0 + j
                        nc.tensor.matmul(
                            ps_o,
                            lhsT=pT[:, j * 128:(j + 1) * 128],
                            rhs=V_sb[:, pr, kt, h2 * 65:(h2 + 1) * 65],
                            start=(j == 0),
                            stop=(j == nk - 1),
                        )
                    recip = osp.tile([128, 1], F32, tag="rc")
                    nc.vector.reciprocal(out=recip, in_=ps_o[:, 64:65])
                    nc.vector.tensor_scalar_mul(
                        out=out_sb[:, h2 * 64:(h2 + 1) * 64],
                        in0=ps_o[:, 0:64],
                        scalar1=recip,
                    )
                ps_t = smp.tile([128, 128], BF16, tag="sm")
                nc.tensor.transpose(out=ps_t, in_=out_sb, identity=ident)
                nc.vector.tensor_copy(
                    out=OT_sb[:, pr, qt * 128:(qt + 1) * 128], in_=ps_t
                )

    # ---------------- phase 3: partial O-projection ----------------
    with (
        tc.tile_pool(name="fo", bufs=4, space="PSUM") as fop,
        tc.tile_pool(name="fs", bufs=4) as fsp,
    ):
        for ot in range(4):
            for ch in range(4):
                ps = fop.tile([128, 512], F32, tag="fo")
                for pr in range(2):
                    nc.tensor.matmul(
                        ps,
                        lhsT=wo_sb[:, pr, ot * 128:(ot + 1) * 128],
                        rhs=OT_sb[:, pr, ch * 512:(ch + 1) * 512],
                        start=(pr == 0),
                        stop=(pr == 1),
                    )
                fs = fsp.tile([128, 512], BF16, tag="fs")
                nc.vector.tensor_copy(out=fs, in_=ps)
                nc.sync.dma_start(
                    out=outT[ot * 128:(ot + 1) * 128, ch * 512:(ch + 1) * 512],
                    in_=fs,
                )


@functools.lru_cache(maxsize=2)
def _build(use_bias=True):
    nc = bacc.Bacc(
        "TRN2", target_bir_lowering=False, debug=False, num_devices=N_CORES
    )
    io = {
        "xT": nc.dram_tensor("xT", [D, S], BF16, kind="ExternalInput").ap(),
        "wq": nc.dram_tensor("wq", [D, 256], BF16, kind="ExternalInput").ap(),
        "wk": nc.dram_tensor("wk", [D, 256], BF16, kind="ExternalInput").ap(),
        "wv": nc.dram_tensor("wv", [D, 256], BF16, kind="ExternalInput").ap(),
        "wo": nc.dram_tensor("wo", [256, D], BF16, kind="ExternalInput").ap(),
        "bq": nc.dram_tensor("bq", [1, 256], BF16, kind="ExternalInput").ap(),
        "bk": nc.dram_tensor("bk", [1, 256], BF16, kind="ExternalInput").ap(),
        "bv": nc.dram_tensor("bv", [1, 256], BF16, kind="ExternalInput").ap(),
        "outT": nc.dram_tensor("outT", [D, S], BF16, kind="ExternalOutput").ap(),
    }
    with tile.TileContext(nc) as tc:
        with ExitStack() as ctx:
            _emit(ctx, tc, io, use_bias)
    nc.compile()
    return nc


def make_in_maps(x, W_Q, b_Q, W_K, b_K, W_V, b_V, W_O, b_O):
    in_maps = []
    for c in range(N_CORES):
        b, hg = c // 2, c % 2
        hs = hg * 256
        in_maps.append(
            {
                "xT": np.ascontiguousarray(x[b].T).astype(NBF),
                "wq": np.ascontiguousarray(W_Q[:, hs:hs + 256]).astype(NBF),
                "wk": np.ascontiguousarray(W_K[:, hs:hs + 256]).astype(NBF),
                "wv": np.ascontiguousarray(W_V[:, hs:hs + 256]).astype(NBF),
                "wo": np.ascontiguousarray(W_O[hs:hs + 256, :]).astype(NBF),
                "bq": b_Q[None, hs:hs + 256].astype(NBF),
                "bk": b_K[None, hs:hs + 256].astype(NBF),
                "bv": b_V[None, hs:hs + 256].astype(NBF),
            }
        )
    return in_maps


def kernel(x, W_Q, b_Q, W_K, b_K, W_V, b_V, W_O, b_O):
    global LAST_RESULTS
    x, W_Q, b_Q, W_K, b_K, W_V, b_V, W_O, b_O = (
        np.asarray(a, dtype=np.float32)
        for a in (x, W_Q, b_Q, W_K, b_K, W_V, b_V, W_O, b_O)
    )
    use_bias = bool(
        np.any(b_Q) or np.any(b_K) or np.any(b_V)
    )  # projection biases are all-zero in this model's inputs
    nc = _build(use_bias)
    in_maps = make_in_maps(x, W_Q, b_Q, W_K, b_K, W_V, b_V, W_O, b_O)
    res = run_bass_kernel_spmd(nc, in_maps, core_ids=list(range(N_CORES)))
    LAST_RESULTS = res
    out = np.empty((4, S, D), np.float32)
    for b in range(4):
        acc = res.results[2 * b]["outT"].astype(np.float32) + res.results[
            2 * b + 1
        ]["outT"].astype(np.float32)
        out[b] = acc.T + b_O[None, :]
    return out


# revision 25
# speedup vs baseline: 1.2460x; 1.0048x over previous
"""Banded sparse attention + MLP projections for TRN2, 8-core SPMD.

Problem: out = (softmax(mask(Q K^T / sqrt(dk))) V) W_O + b_O with
Q/K/V = x W_{Q,K,V} + b, x:[4, 2048, 512], 8 heads, dk=64.

The "log-sparse + k neighbors" mask with k = S//2 = 1024 degenerates to a
banded causal mask: valid iff 0 <= i - j <= 1024 (powers of 2 above 1024
exceed the max distance 2047... the next power is 2048 which is out of
range).  So each 128-query tile attends to at most 9 key tiles.

Sharding: 8 cores = 4 batches x 2 head-groups (4 heads each).  Each core
computes its heads' Q^T/K^T/V projections, banded attention in a
scores-transposed layout (kpos on partitions), and a partial O-projection
outT = W_O[heads].T @ attn_out^T of shape [512, 2048].  Host sums the two
half-partials per batch, transposes, and adds b_O.

All matmuls run in bf16 (fp32 PSUM accumulation); measured end-to-end
scale-relative absmax error vs the fp32 reference is ~3e-3.
"""

import functools
from contextlib import ExitStack

import numpy as np
import ml_dtypes

import concourse.bacc as bacc
import concourse.mybir as mybir
import concourse.tile as tile
from concourse.bass_utils import run_bass_kernel_spmd
from concourse.masks import make_identity, make_upper_triangular, make_lower_triangular

BF16 = mybir.dt.bfloat16
F32 = mybir.dt.float32
NBF = ml_dtypes.bfloat16

S, D = 2048, 512
NT = S // 128          # 16 token tiles
MAXNK = 9              # max key tiles in the band per query tile
N_CORES = 8

LAST_RESULTS = None    # BassKernelResults of the most recent run (for profiling)


def _emit(ctx: ExitStack, tc, io, use_bias):
    nc = tc.nc
    xT, wq, wk, wv, wo, bq, bk, bv, outT = (
        io[k] for k in ("xT", "wq", "wk", "wv", "wo", "bq", "bk", "bv", "outT")
    )

    persist = ctx.enter_context(tc.tile_pool(name="persist", bufs=1))

    ident = persist.tile([128, 128], BF16)
    make_identity(nc, ident)
    # scores are held transposed: [kpos (partition), q (free)].
    # diag tile valid iff q >= k  -> upper triangular incl diag
    # left band-edge tile valid iff q <= k -> lower triangular incl diag
    m_diag = persist.tile([128, 128], BF16)
    make_upper_triangular(nc, m_diag, val=1.0, diag=True)
    m_left = persist.tile([128, 128], BF16)
    make_lower_triangular(nc, m_left, val=1.0, diag=True)
    ones_row = persist.tile([1, 512], BF16)
    nc.vector.memset(ones_row, 1.0)

    xT_sb = persist.tile([128, 4, S], BF16)
    wq_sb = persist.tile([128, 4, 256], BF16)
    wk_sb = persist.tile([128, 4, 256], BF16)
    wv_sb = persist.tile([128, 4, 256], BF16)
    for kt in range(4):
        nc.sync.dma_start(out=xT_sb[:, kt, :], in_=xT[kt * 128:(kt + 1) * 128, :])
        nc.sync.dma_start(out=wq_sb[:, kt, :], in_=wq[kt * 128:(kt + 1) * 128, :])
        nc.sync.dma_start(out=wk_sb[:, kt, :], in_=wk[kt * 128:(kt + 1) * 128, :])
        nc.sync.dma_start(out=wv_sb[:, kt, :], in_=wv[kt * 128:(kt + 1) * 128, :])
    wo_sb = persist.tile([128, 2, 512], BF16)
    for pr in range(2):
        nc.sync.dma_start(out=wo_sb[:, pr, :], in_=wo[pr * 128:(pr + 1) * 128, :])
    bq_sb = persist.tile([1, 256], BF16)
    bk_sb = persist.tile([1, 256], BF16)
    bv_sb = persist.tile([1, 256], BF16)
    nc.sync.dma_start(out=bq_sb, in_=bq[:, :])
    nc.sync.dma_start(out=bk_sb, in_=bk[:, :])
    nc.sync.dma_start(out=bv_sb, in_=bv[:, :])

    # Q^T / K^T per head pair: rows 0-63 head A dims, 64-127 head B dims.
    QT_sb = persist.tile([128, 2, S], BF16)
    KT_sb = persist.tile([128, 2, S], BF16)
    # V in [token, d] layout per k-tile, stored as [dA0..dA63, onesA,
    # dB0..dB63, onesB] so [V_h | ones] is one contiguous [128, 65] slice.
    V_sb = persist.tile([128, 2, NT, 130], BF16)
    nc.gpsimd.memset(V_sb[:, :, :, 64:65], 1.0)
    nc.gpsimd.memset(V_sb[:, :, :, 129:130], 1.0)
    # normalized attention output, transposed: rows = head dims of the pair
    OT_sb = persist.tile([128, 2, S], BF16)

    # ---------------- phase 1: projections ----------------
    with tc.tile_pool(name="pj", bufs=4, space="PSUM") as pj:
        for pr in range(2):
            for w_sb, b_sb, dst in ((wq_sb, bq_sb, QT_sb), (wk_sb, bk_sb, KT_sb)):
                for ch in range(4):
                    ps = pj.tile([128, 512], F32, tag="pjq")
                    for kt in range(4):
                        nc.tensor.matmul(
                            ps,
                            lhsT=w_sb[:, kt, pr * 128:(pr + 1) * 128],
                            rhs=xT_sb[:, kt, ch * 512:(ch + 1) * 512],
                            start=(kt == 0),
                            stop=(kt == 3 and not use_bias),
                        )
                    if use_bias:
                        # bias as a K=1 rank-1 update: b[m] * ones[n]
                        nc.tensor.matmul(
                            ps,
                            lhsT=b_sb[:, pr * 128:(pr + 1) * 128],
                            rhs=ones_row,
                            start=False,
                            stop=True,
                        )
                    nc.vector.tensor_copy(
                        out=dst[:, pr, ch * 512:(ch + 1) * 512], in_=ps
                    )
        for tt in range(NT):
            ps = pj.tile([128, 256], F32, tag="pjv")
            for kt in range(4):
                nc.tensor.matmul(
                    ps,
                    lhsT=xT_sb[:, kt, tt * 128:(tt + 1) * 128],
                    rhs=wv_sb[:, kt, 0:256],
                    start=(kt == 0),
                    stop=(kt == 3 and not use_bias),
                )
            if use_bias:
                nc.tensor.matmul(
                    ps,
                    lhsT=ones_row[:, 0:128],
                    rhs=bv_sb[:, 0:256],
                    start=False,
                    stop=True,
                )
            # interleave each pair's head halves into its 65-col blocks
            for pr in range(2):
                src = ps[:, pr * 128:(pr + 1) * 128].rearrange(
                    "p (two d) -> p two d", two=2
                )
                dstv = V_sb[:, pr, tt, 0:130].rearrange(
                    "p (two dp) -> p two dp", two=2
                )[:, :, 0:64]
                nc.vector.tensor_copy(out=dstv, in_=src)

    # ---------------- phase 2: banded attention ----------------
    with (
        tc.tile_pool(name="sc", bufs=2, space="PSUM") as scp,
        tc.tile_pool(name="sm", bufs=2, space="PSUM") as smp,
        tc.tile_pool(name="pt", bufs=4) as ptp,
        tc.tile_pool(name="os", bufs=4) as osp,
    ):
        for pr in range(2):
            for qt in range(NT):
                nk = min(qt + 1, MAXNK)
                kt0 = qt - nk + 1
                ps_s = [
                    scp.tile([128, nk * 128], F32, tag="s", name=f"s{h2}")
                    for h2 in range(2)
                ]
                # scores^T: stationary K^T k-tile, moving Q^T q-tile.
                # h2=0 uses array rows 0-63, h2=1 rows 64-127 (concurrent).
                for j in range(nk):
                    kt = kt0 + j
                    for h2 in range(2):
                        lo, hi = h2 * 64, h2 * 64 + 64
                        nc.tensor.matmul(
                            ps_s[h2][:, j * 128:(j + 1) * 128],
                            lhsT=KT_sb[lo:hi, pr, kt * 128:(kt + 1) * 128],
                            rhs=QT_sb[lo:hi, pr, qt * 128:(qt + 1) * 128],
                            start=True,
                            stop=True,
                        )
                out_sb = osp.tile([128, 128], BF16, tag="ob")
                for h2 in range(2):
                    pT = ptp.tile([128, MAXNK * 128], BF16, tag="pt")
                    nc.scalar.activation(
                        out=pT[:, 0:nk * 128],
                        in_=ps_s[h2][:, 0:nk * 128],
                        func=mybir.ActivationFunctionType.Exp,
                        scale=0.125,  # 1/sqrt(dk)
                    )
                    if qt >= MAXNK - 1:
                        nc.gpsimd.tensor_mul(
                            out=pT[:, 0:128], in0=pT[:, 0:128], in1=m_left
                        )
                    nc.gpsimd.tensor_mul(
                        out=pT[:, (nk - 1) * 128:nk * 128],
                        in0=pT[:, (nk - 1) * 128:nk * 128],
                        in1=m_diag,
                    )
                    # out^T accumulate: stationary p^T tile, moving [V|ones].
                    # col 64 of the result is the softmax denominator.
                    ps_o = smp.tile([128, 65], F32, tag="sm")
                    for j in range(nk):
                        kt = kt0 + j
                        nc.tensor.matmul(
                            ps_o,
                            lhsT=pT[:, j * 128:(j + 1) * 128],
                            rhs=V_sb[:, pr, kt, h2 * 65:(h2 + 1) * 65],
                            start=(j == 0),
                            stop=(j == nk - 1),
                        )
                    recip = osp.tile([128, 1], F32, tag="rc")
                    nc.vector.reciprocal(out=recip, in_=ps_o[:, 64:65])
                    nc.vector.tensor_scalar_mul(
                        out=out_sb[:, h2 * 64:(h2 + 1) * 64],
                        in0=ps_o[:, 0:64],
                        scalar1=recip,
                    )
                ps_t = smp.tile([128, 128], BF16, tag="sm")
                nc.tensor.transpose(out=ps_t, in_=out_sb, identity=ident)
                nc.vector.tensor_copy(
                    out=OT_sb[:, pr, qt * 128:(qt + 1) * 128], in_=ps_t
                )

    # ---------------- phase 3: partial O-projection ----------------
    with (
        tc.tile_pool(name="fo", bufs=4, space="PSUM") as fop,
        tc.tile_pool(name="fs", bufs=4) as fsp,
    ):
        for ot in range(4):
            for ch in range(4):
                ps = fop.tile([128, 512], F32, tag="fo")
                for pr in range(2):
                    nc.tensor.matmul(
                        ps,
                        lhsT=wo_sb[:, pr, ot * 128:(ot + 1) * 128],
                        rhs=OT_sb[:, pr, ch * 512:(ch + 1) * 512],
                        start=(pr == 0),
                        stop=(pr == 1),
                    )
                fs = fsp.tile([128, 512], BF16, tag="fs")
                nc.vector.tensor_copy(out=fs, in_=ps)
                nc.sync.dma_start(
                    out=outT[ot * 128:(ot + 1) * 128, ch * 512:(ch + 1) * 512],
                    in_=fs,
                )


@functools.lru_cache(maxsize=2)
def _build(use_bias=True):
    nc = bacc.Bacc(
        "TRN2", target_bir_lowering=False, debug=False, num_devices=N_CORES
    )
    io = {
        "xT": nc.dram_tensor("xT", [D, S], BF16, kind="ExternalInput").ap(),
        "wq": nc.dram_tensor("wq", [D, 256], BF16, kind="ExternalInput").ap(),
        "wk": nc.dram_tensor("wk", [D, 256], BF16, kind="ExternalInput").ap(),
        "wv": nc.dram_tensor("wv", [D, 256], BF16, kind="ExternalInput").ap(),
        "wo": nc.dram_tensor("wo", [256, D], BF16, kind="ExternalInput").ap(),
        "bq": nc.dram_tensor("bq", [1, 256], BF16, kind="ExternalInput").ap(),
        "bk": nc.dram_tensor("bk", [1, 256], BF16, kind="ExternalInput").ap(),
        "bv": nc.dram_tensor("bv", [1, 256], BF16, kind="ExternalInput").ap(),
        "outT": nc.dram_tensor("outT", [D, S], BF16, kind="ExternalOutput").ap(),
    }
    with tile.TileContext(nc) as tc:
        with ExitStack() as ctx:
            _emit(ctx, tc, io, use_bias)
    nc.compile()
    return nc


def make_in_maps(x, W_Q, b_Q, W_K, b_K, W_V, b_V, W_O, b_O):
    in_maps = []
    for c in range(N_CORES):
        b, hg = c // 2, c % 2
        hs = hg * 256
        in_maps.append(
            {
                "xT": np.ascontiguousarray(x[b].T).astype(NBF),
                "wq": np.ascontiguousarray(W_Q[:, hs:hs + 256]).astype(NBF),
                "wk": np.ascontiguousarray(W_K[:, hs:hs + 256]).astype(NBF),
                "wv": np.ascontiguousarray(W_V[:, hs:hs + 256]).astype(NBF),
                "wo": np.ascontiguousarray(W_O[hs:hs + 256, :]).astype(NBF),
                "bq": b_Q[None, hs:hs + 256].astype(NBF),
                "bk": b_K[None, hs:hs + 256].astype(NBF),
                "bv": b_V[None, hs:hs + 256].astype(NBF),
            }
        )
    return in_maps


def kernel(x, W_Q, b_Q, W_K, b_K, W_V, b_V, W_O, b_O):
    global LAST_RESULTS
    x, W_Q, b_Q, W_K, b_K, W_V, b_V, W_O, b_O = (
        np.asarray(a, dtype=np.float32)
        for a in (x, W_Q, b_Q, W_K, b_K, W_V, b_V, W_O, b_O)
    )
    use_bias = bool(
        np.any(b_Q) or np.any(b_K) or np.any(b_V)
    )  # projection biases are all-zero in this model's inputs
    nc = _build(use_bias)
    in_maps = make_in_maps(x, W_Q, b_Q, W_K, b_K, W_V, b_V, W_O, b_O)
    res = run_bass_kernel_spmd(nc, in_maps, core_ids=list(range(N_CORES)))
    LAST_RESULTS = res
    out = np.empty((4, S, D), np.float32)
    for b in range(4):
        acc = res.results[2 * b]["outT"].astype(np.float32) + res.results[
            2 * b + 1
        ]["outT"].astype(np.float32)
        out[b] = acc.T + b_O[None, :]
    return out


# revision 26
# speedup vs baseline: 1.2492x; 1.0026x over previous
"""Banded sparse attention + MLP projections for TRN2, 8-core SPMD.

Problem: out = (softmax(mask(Q K^T / sqrt(dk))) V) W_O + b_O with
Q/K/V = x W_{Q,K,V} + b, x:[4, 2048, 512], 8 heads, dk=64.

The "log-sparse + k neighbors" mask with k = S//2 = 1024 degenerates to a
banded causal mask: valid iff 0 <= i - j <= 1024 (powers of 2 above 1024
exceed the max distance 2047... the next power is 2048 which is out of
range).  So each 128-query tile attends to at most 9 key tiles.

Sharding: 8 cores = 4 batches x 2 head-groups (4 heads each).  Each core
computes its heads' Q^T/K^T/V projections, banded attention in a
scores-transposed layout (kpos on partitions), and a partial O-projection
outT = W_O[heads].T @ attn_out^T of shape [512, 2048].  Host sums the two
half-partials per batch, transposes, and adds b_O.

All matmuls run in bf16 (fp32 PSUM accumulation); measured end-to-end
scale-relative absmax error vs the fp32 reference is ~3e-3.
"""

import functools
from contextlib import ExitStack

import numpy as np
import ml_dtypes

import concourse.bacc as bacc
import concourse.mybir as mybir
import concourse.tile as tile
from concourse.bass_utils import run_bass_kernel_spmd
from concourse.masks import make_identity, make_upper_triangular, make_lower_triangular

BF16 = mybir.dt.bfloat16
F32 = mybir.dt.float32
NBF = ml_dtypes.bfloat16

S, D = 2048, 512
NT = S // 128          # 16 token tiles
MAXNK = 9              # max key tiles in the band per query tile
N_CORES = 8

LAST_RESULTS = None    # BassKernelResults of the most recent run (for profiling)


def _emit(ctx: ExitStack, tc, io, use_bias):
    nc = tc.nc
    xT, wq, wk, wv, wo, bq, bk, bv, outT = (
        io[k] for k in ("xT", "wq", "wk", "wv", "wo", "bq", "bk", "bv", "outT")
    )

    persist = ctx.enter_context(tc.tile_pool(name="persist", bufs=1))

    ident = persist.tile([128, 128], BF16)
    make_identity(nc, ident)
    # scores are held transposed: [kpos (partition), q (free)].
    # diag tile valid iff q >= k  -> upper triangular incl diag
    # left band-edge tile valid iff q <= k -> lower triangular incl diag
    m_diag = persist.tile([128, 128], BF16)
    make_upper_triangular(nc, m_diag, val=1.0, diag=True)
    m_left = persist.tile([128, 128], BF16)
    make_lower_triangular(nc, m_left, val=1.0, diag=True)
    ones_row = persist.tile([1, 512], BF16)
    nc.vector.memset(ones_row, 1.0)

    xT_sb = persist.tile([128, 4, S], BF16)
    wq_sb = persist.tile([128, 4, 256], BF16)
    wk_sb = persist.tile([128, 4, 256], BF16)
    wv_sb = persist.tile([128, 4, 256], BF16)
    for kt in range(4):
        nc.sync.dma_start(out=xT_sb[:, kt, :], in_=xT[kt * 128:(kt + 1) * 128, :])
        nc.sync.dma_start(out=wq_sb[:, kt, :], in_=wq[kt * 128:(kt + 1) * 128, :])
        nc.sync.dma_start(out=wk_sb[:, kt, :], in_=wk[kt * 128:(kt + 1) * 128, :])
        nc.sync.dma_start(out=wv_sb[:, kt, :], in_=wv[kt * 128:(kt + 1) * 128, :])
    wo_sb = persist.tile([128, 2, 512], BF16)
    for pr in range(2):
        nc.sync.dma_start(out=wo_sb[:, pr, :], in_=wo[pr * 128:(pr + 1) * 128, :])
    bq_sb = persist.tile([1, 256], BF16)
    bk_sb = persist.tile([1, 256], BF16)
    bv_sb = persist.tile([1, 256], BF16)
    nc.sync.dma_start(out=bq_sb, in_=bq[:, :])
    nc.sync.dma_start(out=bk_sb, in_=bk[:, :])
    nc.sync.dma_start(out=bv_sb, in_=bv[:, :])

    # Q^T / K^T per head pair: rows 0-63 head A dims, 64-127 head B dims.
    QT_sb = persist.tile([128, 2, S], BF16)
    KT_sb = persist.tile([128, 2, S], BF16)
    # V in [token, d] layout per k-tile, stored as [dA0..dA63, onesA,
    # dB0..dB63, onesB] so [V_h | ones] is one contiguous [128, 65] slice.
    V_sb = persist.tile([128, 2, NT, 130], BF16)
    nc.gpsimd.memset(V_sb[:, :, :, 64:65], 1.0)
    nc.gpsimd.memset(V_sb[:, :, :, 129:130], 1.0)
    # normalized attention output, transposed: rows = head dims of the pair
    OT_sb = persist.tile([128, 2, S], BF16)

    # ---------------- phase 1: projections ----------------
    with tc.tile_pool(name="pj", bufs=4, space="PSUM") as pj:
        for pr in range(2):
            for w_sb, b_sb, dst in ((wq_sb, bq_sb, QT_sb), (wk_sb, bk_sb, KT_sb)):
                for ch in range(4):
                    ps = pj.tile([128, 512], F32, tag="pjq")
                    for kt in range(4):
                        nc.tensor.matmul(
                            ps,
                            lhsT=w_sb[:, kt, pr * 128:(pr + 1) * 128],
                            rhs=xT_sb[:, kt, ch * 512:(ch + 1) * 512],
                            start=(kt == 0),
                            stop=(kt == 3 and not use_bias),
                        )
                    if use_bias:
                        # bias as a K=1 rank-1 update: b[m] * ones[n]
                        nc.tensor.matmul(
                            ps,
                            lhsT=b_sb[:, pr * 128:(pr + 1) * 128],
                            rhs=ones_row,
                            start=False,
                            stop=True,
                        )
                    nc.vector.tensor_copy(
                        out=dst[:, pr, ch * 512:(ch + 1) * 512], in_=ps
                    )
        for tt in range(NT):
            ps = pj.tile([128, 256], F32, tag="pjv")
            for kt in range(4):
                nc.tensor.matmul(
                    ps,
                    lhsT=xT_sb[:, kt, tt * 128:(tt + 1) * 128],
                    rhs=wv_sb[:, kt, 0:256],
                    start=(kt == 0),
                    stop=(kt == 3 and not use_bias),
                )
            if use_bias:
                nc.tensor.matmul(
                    ps,
                    lhsT=ones_row[:, 0:128],
                    rhs=bv_sb[:, 0:256],
                    start=False,
                    stop=True,
                )
            # interleave each pair's head halves into its 65-col blocks
            for pr in range(2):
                src = ps[:, pr * 128:(pr + 1) * 128].rearrange(
                    "p (two d) -> p two d", two=2
                )
                dstv = V_sb[:, pr, tt, 0:130].rearrange(
                    "p (two dp) -> p two dp", two=2
                )[:, :, 0:64]
                nc.vector.tensor_copy(out=dstv, in_=src)

    # ---------------- phase 2: banded attention ----------------
    with (
        tc.tile_pool(name="sc", bufs=2, space="PSUM") as scp,
        tc.tile_pool(name="sm", bufs=2, space="PSUM") as smp,
        tc.tile_pool(name="pt", bufs=6) as ptp,
        tc.tile_pool(name="os", bufs=6) as osp,
    ):
        for pr in range(2):
            for qt in range(NT):
                nk = min(qt + 1, MAXNK)
                kt0 = qt - nk + 1
                ps_s = [
                    scp.tile([128, nk * 128], F32, tag="s", name=f"s{h2}")
                    for h2 in range(2)
                ]
                # scores^T: stationary K^T k-tile, moving Q^T q-tile.
                # h2=0 uses array rows 0-63, h2=1 rows 64-127 (concurrent).
                for j in range(nk):
                    kt = kt0 + j
                    for h2 in range(2):
                        lo, hi = h2 * 64, h2 * 64 + 64
                        nc.tensor.matmul(
                            ps_s[h2][:, j * 128:(j + 1) * 128],
                            lhsT=KT_sb[lo:hi, pr, kt * 128:(kt + 1) * 128],
                            rhs=QT_sb[lo:hi, pr, qt * 128:(qt + 1) * 128],
                            start=True,
                            stop=True,
                        )
                out_sb = osp.tile([128, 128], BF16, tag="ob")
                for h2 in range(2):
                    pT = ptp.tile([128, MAXNK * 128], BF16, tag="pt")
                    nc.scalar.activation(
                        out=pT[:, 0:nk * 128],
                        in_=ps_s[h2][:, 0:nk * 128],
                        func=mybir.ActivationFunctionType.Exp,
                        scale=0.125,  # 1/sqrt(dk)
                    )
                    if qt >= MAXNK - 1:
                        nc.gpsimd.tensor_mul(
                            out=pT[:, 0:128], in0=pT[:, 0:128], in1=m_left
                        )
                    nc.gpsimd.tensor_mul(
                        out=pT[:, (nk - 1) * 128:nk * 128],
                        in0=pT[:, (nk - 1) * 128:nk * 128],
                        in1=m_diag,
                    )
                    # out^T accumulate: stationary p^T tile, moving [V|ones].
                    # col 64 of the result is the softmax denominator.
                    ps_o = smp.tile([128, 65], F32, tag="sm")
                    for j in range(nk):
                        kt = kt0 + j
                        nc.tensor.matmul(
                            ps_o,
                            lhsT=pT[:, j * 128:(j + 1) * 128],
                            rhs=V_sb[:, pr, kt, h2 * 65:(h2 + 1) * 65],
                            start=(j == 0),
                            stop=(j == nk - 1),
                        )
                    recip = osp.tile([128, 1], F32, tag="rc")
                    nc.vector.reciprocal(out=recip, in_=ps_o[:, 64:65])
                    nc.vector.tensor_scalar_mul(
                        out=out_sb[:, h2 * 64:(h2 + 1) * 64],
                        in0=ps_o[:, 0:64],
                        scalar1=recip,
                    )
                ps_t = smp.tile([128, 128], BF16, tag="sm")
                nc.tensor.transpose(out=ps_t, in_=out_sb, identity=ident)
                nc.vector.tensor_copy(
                    out=OT_sb[:, pr, qt * 128:(qt + 1) * 128], in_=ps_t
                )

    # ---------------- phase 3: partial O-projection ----------------
    with (
        tc.tile_pool(name="fo", bufs=4, space="PSUM") as fop,
        tc.tile_pool(name="fs", bufs=6) as fsp,
    ):
        for ot in range(4):
            for ch in range(4):
                ps = fop.tile([128, 512], F32, tag="fo")
                for pr in range(2):
                    nc.tensor.matmul(
                        ps,
                        lhsT=wo_sb[:, pr, ot * 128:(ot + 1) * 128],
                        rhs=OT_sb[:, pr, ch * 512:(ch + 1) * 512],
                        start=(pr == 0),
                        stop=(pr == 1),
                    )
                fs = fsp.tile([128, 512], BF16, tag="fs")
                nc.vector.tensor_copy(out=fs, in_=ps)
                nc.sync.dma_start(
                    out=outT[ot * 128:(ot + 1) * 128, ch * 512:(ch + 1) * 512],
                    in_=fs,
                )


@functools.lru_cache(maxsize=2)
def _build(use_bias=True):
    nc = bacc.Bacc(
        "TRN2", target_bir_lowering=False, debug=False, num_devices=N_CORES
    )
    io = {
        "xT": nc.dram_tensor("xT", [D, S], BF16, kind="ExternalInput").ap(),
        "wq": nc.dram_tensor("wq", [D, 256], BF16, kind="ExternalInput").ap(),
        "wk": nc.dram_tensor("wk", [D, 256], BF16, kind="ExternalInput").ap(),
        "wv": nc.dram_tensor("wv", [D, 256], BF16, kind="ExternalInput").ap(),
        "wo": nc.dram_tensor("wo", [256, D], BF16, kind="ExternalInput").ap(),
        "bq": nc.dram_tensor("bq", [1, 256], BF16, kind="ExternalInput").ap(),
        "bk": nc.dram_tensor("bk", [1, 256], BF16, kind="ExternalInput").ap(),
        "bv": nc.dram_tensor("bv", [1, 256], BF16, kind="ExternalInput").ap(),
        "outT": nc.dram_tensor("outT", [D, S], BF16, kind="ExternalOutput").ap(),
    }
    with tile.TileContext(nc) as tc:
        with ExitStack() as ctx:
            _emit(ctx, tc, io, use_bias)
    nc.compile()
    return nc


def make_in_maps(x, W_Q, b_Q, W_K, b_K, W_V, b_V, W_O, b_O):
    in_maps = []
    for c in range(N_CORES):
        b, hg = c // 2, c % 2
        hs = hg * 256
        in_maps.append(
            {
                "xT": np.ascontiguousarray(x[b].T).astype(NBF),
                "wq": np.ascontiguousarray(W_Q[:, hs:hs + 256]).astype(NBF),
                "wk": np.ascontiguousarray(W_K[:, hs:hs + 256]).astype(NBF),
                "wv": np.ascontiguousarray(W_V[:, hs:hs + 256]).astype(NBF),
                "wo": np.ascontiguousarray(W_O[hs:hs + 256, :]).astype(NBF),
                "bq": b_Q[None, hs:hs + 256].astype(NBF),
                "bk": b_K[None, hs:hs + 256].astype(NBF),
                "bv": b_V[None, hs:hs + 256].astype(NBF),
            }
        )
    return in_maps


def kernel(x, W_Q, b_Q, W_K, b_K, W_V, b_V, W_O, b_O):
    global LAST_RESULTS
    x, W_Q, b_Q, W_K, b_K, W_V, b_V, W_O, b_O = (
        np.asarray(a, dtype=np.float32)
        for a in (x, W_Q, b_Q, W_K, b_K, W_V, b_V, W_O, b_O)
    )
    use_bias = bool(
        np.any(b_Q) or np.any(b_K) or np.any(b_V)
    )  # projection biases are all-zero in this model's inputs
    nc = _build(use_bias)
    in_maps = make_in_maps(x, W_Q, b_Q, W_K, b_K, W_V, b_V, W_O, b_O)
    res = run_bass_kernel_spmd(nc, in_maps, core_ids=list(range(N_CORES)))
    LAST_RESULTS = res
    out = np.empty((4, S, D), np.float32)
    for b in range(4):
        acc = res.results[2 * b]["outT"].astype(np.float32) + res.results[
            2 * b + 1
        ]["outT"].astype(np.float32)
        out[b] = acc.T + b_O[None, :]
    return out
